# revision 77
# baseline (speedup 1.0000x reference)
"""Multi-head causal attention (B=4, T=2048, C=1024, H=16, HD=64) on 8 TRN2 NeuronCores.

Sharding: core c handles batch b = c//2 and heads hg*8..hg*8+8 where hg = c%2
(data parallel on B, tensor parallel on heads). Each core computes
qkv projection for its head group, causal attention for its 8 heads, and a
partial output projection over its 512 local channels. Host sums the two
partial projections per batch and adds the bias.

Attention is computed transposed (S.T = K.T-stationary @ Q-moving) so that
softmax'd probabilities P.T come out of the exp directly in the layout the
AV product wants as its *stationary* operand:
  - per (head, key-row-block jt): S.T[j, i] for i >= jt*128 in psum chunks,
    causal mask added on the diagonal 128-block by GpSimd, exp on ACT into
    bf16 P.T chunks (no transposes anywhere).
  - AV: out[i-tile, 65] += P.T-block (stationary) @ [V_head | 1] (moving)
    accumulated over jt in 16 per-head psum accumulators; the ones column
    yields the softmax row-sums for free.
  - normalize: 1/rowsum (DVE) * O (GpSimd) -> bf16, DMA-transposed into
    oT[d, t] for the output projection.
qkv projection / output projection matmul groups are interleaved between
attention chunks as PE filler, paced by an emission-time cost estimate, so
the PE never starves while ACT drains the exps. Same-engine producers are
force-emitted before their consumers (engines execute in order).

Per-core device layouts (all chosen so no on-chip transposes of x/W needed):
  xT    [C=1024, T=2048] bf16   (x[b].T, host-transposed)
  wqkT  [C=1024, 1024]   bf16   (rows: Q of 8 heads then K of 8 heads, transposed;
                                 Q part pre-scaled by HD^-0.5)
  wvT   [C=1024, 512]    bf16
  wpT   [512, 1024]      bf16   (W_proj columns for local channels, transposed)
"""

import numpy as np
import ml_dtypes

B, T, C = 4, 2048, 1024
H = 16
HD = 64
NCORES = 8
P = 128

_CACHE = {}

# cost-model constants for emission-time pacing (ns)
_PE_CYC = 1.0 / 2.4
_ACT_CYC = 1.0 / 1.2
_ACT_OVH = 250.0


def _build_program():
    import concourse.bass as bass
    import concourse.mybir as mybir
    import concourse.tile as tile
    from concourse import bacc
    from concourse.masks import make_lower_triangular, make_identity
    from contextlib import ExitStack

    DT_BF = mybir.dt.bfloat16
    DT_F32 = mybir.dt.float32
    Exp = mybir.ActivationFunctionType.Exp

    nc = bacc.Bacc("TRN2", target_bir_lowering=False, num_devices=NCORES)
    xT = nc.dram_tensor("xT", [C, T], DT_BF, kind="ExternalInput")
    wqkT = nc.dram_tensor("wqkT", [C, 1024], DT_BF, kind="ExternalInput")
    wvT = nc.dram_tensor("wvT", [C, 512], DT_BF, kind="ExternalInput")
    wpT = nc.dram_tensor("wpT", [512, 1024], DT_BF, kind="ExternalInput")
    # y is bf16: halves the store traffic; the host sums the two per-batch
    # partials in fp32 (the partials themselves are bf16-rounded anyway)
    y = nc.dram_tensor("y", [T, C], DT_BF, kind="ExternalOutput")

    NT = T // P            # 16 t-tiles
    NH = 8                 # heads per core

    with tile.TileContext(nc) as tc, ExitStack() as ctx:
        pers = ctx.enter_context(tc.tile_pool(name="pers", bufs=1))
        ptp = ctx.enter_context(tc.tile_pool(name="ptp", bufs=3))
        recp = ctx.enter_context(tc.tile_pool(name="recp", bufs=4))
        worky = ctx.enter_context(tc.tile_pool(name="worky", bufs=8))
        # PSUM banks: stA 2 + stB 1 + av 3 + acc 2 = 8
        st_ps = ctx.enter_context(tc.tile_pool(name="st", bufs=1, space="PSUM"))
        av_ps = ctx.enter_context(tc.tile_pool(name="av", bufs=1, space="PSUM"))
        acc_ps = ctx.enter_context(tc.tile_pool(name="acc", bufs=2, space="PSUM"))

        # ---- persistent tiles ----
        xt0 = pers.tile([P, 4, T], DT_BF, tag="xt0")
        xt1 = pers.tile([P, 4, T], DT_BF, tag="xt1")

        def xt(cc):
            return (xt0 if cc < 4 else xt1)[:, cc % 4, :]

        wqk = pers.tile([P, 8, 1024], DT_BF, tag="wqk")
        wv = pers.tile([P, 8, 512], DT_BF, tag="wv")
        wp = pers.tile([P, 4, 1024], DT_BF, tag="wp")
        qkT = pers.tile([P, 8, T], DT_BF, tag="qkT")
        v65 = pers.tile([P, NT, NH, 65], DT_BF, tag="v65")
        oT = pers.tile([P, 4, T], DT_BF, tag="oT")
        # normalized-O staging for the in-flight head pair: [t, it, d-pair]
        ostg = pers.tile([P, NT, P], DT_BF, tag="ostg")
        # causal mask + identity (bf16): the mask add rides the PE as
        # ident.T @ maskT accumulated into the S.T psum diagonal block
        maskT = pers.tile([P, P], DT_BF, tag="maskT")
        ident = pers.tile([P, P], DT_BF, tag="ident")
        # head 7's P.T rows, ALL exp'd early (during heads with ACT slack) so
        # the last head needs no ACT at all. Rows 4..15 live in their own
        # tile; rows 0..3 (the big ones) alias xt0's slot, which is dead once
        # every qkv-projection group has run.
        PT7D_OFF = [0]
        for j in range(5, NT):
            PT7D_OFF.append(PT7D_OFF[-1] + T - (j - 1) * P)
        pt7d = pers.tile([P, PT7D_OFF[-1] + P], DT_BF, tag="pt7d")
        pt7a_cache = []

        def pt7row(jt):
            if jt < 4:
                if not pt7a_cache:
                    # one-time realias of xt0's slot (allocated once so all
                    # P.T writes/reads refer to the same logical tile)
                    a = pers.tile([P, 4, T], DT_BF, tag="xt0")
                    pt7a_cache.append(a[:].rearrange("p a t -> p (a t)"))
                return pt7a_cache[0], jt * T - P * (jt * (jt - 1) // 2)
            return pt7d[:], PT7D_OFF[jt - 4]

        stA = st_ps.tile([P, 1024], DT_F32, tag="stA")
        stB = st_ps.tile([P, 512], DT_F32, tag="stB")
        avA = av_ps.tile([P, 7, 65], DT_F32, tag="avA")
        avB = av_ps.tile([P, 7, 65], DT_F32, tag="avB")
        avC = av_ps.tile([P, 2, 65], DT_F32, tag="avC")

        def av_slot(it):
            if it < 7:
                return avA[:, it, :]
            if it < 14:
                return avB[:, it - 7, :]
            return avC[:, it - 14, :]

        # ---- loads ----
        xtre = xT.rearrange("(o p) t -> p o t", p=P)
        wqkre = wqkT.rearrange("(o p) f -> p o f", p=P)
        wvre = wvT.rearrange("(o p) f -> p o f", p=P)
        wpre = wpT.rearrange("(o p) f -> p o f", p=P)
        # weights for pair-0 q/k first, then x in halves; few big DMAs (HWDGE
        # issue costs ~630ns apiece, so batch)
        # NB: qkv matmul chains contract over all 8 c-chunks, so weight loads
        # are split by OUTPUT columns (w*[:, :, osl]), never by c-chunk.
        nc.sync.dma_start(wqk[:, :, 512:768], wqkre[:, :, 512:768])    # oc 4,5
        for tq in range(4):
            tsl = slice(tq * 512, (tq + 1) * 512)
            nc.sync.dma_start(xt0[:, :, tsl], xtre[:, 0:4, tsl])
            nc.sync.dma_start(xt1[:, :, tsl], xtre[:, 4:8, tsl])
            if tq == 0:
                nc.sync.dma_start(wqk[:, :, 0:256], wqkre[:, :, 0:256])  # 0,1
            if tq == 1:
                nc.sync.dma_start(wv[:, :, 0:256], wvre[:, :, 0:256])
        nc.sync.dma_start(wqk[:, :, 768:1024], wqkre[:, :, 768:1024])  # oc 6,7
        nc.sync.dma_start(wqk[:, :, 256:512], wqkre[:, :, 256:512])    # oc 2,3
        nc.sync.dma_start(wv[:, :, 256:512], wvre[:, :, 256:512])
        nc.sync.dma_start(wp[:, :, :], wpre[:, 0:4, :])

        # S.T-layout causal mask: -1e30 where j > i (strict lower triangle)
        make_lower_triangular(nc, maskT[:], val=-1e30, diag=False)
        make_identity(nc, ident[:])
        # ones column of [V | 1] (rowsum accumulator input)
        nc.vector.memset(v65[:, :, :, 64:65], 1.0)

        # ---- work units: qkv projection / output projection groups ----
        def qk_go(oc, tc4):
            ts = slice(tc4 * 512, (tc4 + 1) * 512)
            acc = acc_ps.tile([P, 512], DT_F32, tag="acc")
            for cc in range(8):
                nc.tensor.matmul(
                    acc[:],
                    wqk[:, cc, oc * P:(oc + 1) * P],
                    xt(cc)[:, ts],
                    start=(cc == 0), stop=(cc == 7),
                )
            nc.vector.tensor_copy(qkT[:, oc, ts], acc[:])

        def v_go(tt, half):
            acc = acc_ps.tile([P, 512], DT_F32, tag="acc")
            for cc in range(8):
                nc.tensor.matmul(
                    acc[:, 0:256],
                    xt(cc)[:, tt * P:(tt + 1) * P],
                    wv[:, cc, half * 256:(half + 1) * 256],
                    start=(cc == 0), stop=(cc == 7),
                )
            src = acc[:, 0:256].rearrange("p (h c) -> p h c", c=64)
            nc.vector.tensor_copy(v65[:, tt, half * 4:(half + 1) * 4, 0:64], src)

        # output projection in two per-pair halves: pairs {0,1} become
        # available after head 3, so they fill the late-middle heads; pairs
        # {2,3} complete during head 7. Partial sums staged in bf16 SBUF.
        ysb1 = pers.tile([P, NT, 2, 512], DT_BF, tag="ysb1")
        tail_slots = []
        tail_slots_tp = []

        def proj_acc():
            """PSUM acc for a projection unit; in the tail (head 7 onward:
            no S.T chunks anymore) rotate over the freed st banks too, so
            the psum-readback latency doesn't serialize the proj stream."""
            if tail_slots:
                state["tidx"] += 1
                sl = tail_slots[state["tidx"] % len(tail_slots)]
                if sl is not None:
                    return sl
            return acc_ps.tile([P, 512], DT_F32, tag="acc", name="acc")

        def vec_or_pool():
            state["vp"] ^= 1
            return nc.vector if state["vp"] else nc.gpsimd

        def pja_go(tt, oc2):
            acc = proj_acc()
            for hc in range(2):
                nc.tensor.matmul(
                    acc[:],
                    oT[:, hc, tt * P:(tt + 1) * P],
                    wp[:, hc, oc2 * 512:(oc2 + 1) * 512],
                    start=(hc == 0), stop=(hc == 1),
                )
            vec_or_pool().tensor_copy(ysb1[:, tt, oc2, :], acc[:])

        def pj2_go(tt, oc2):
            force(("pja", tt, oc2))
            acc = proj_acc()
            nc.tensor.matmul(
                acc[:],
                oT[:, 2, tt * P:(tt + 1) * P],
                wp[:, 2, oc2 * 512:(oc2 + 1) * 512],
                start=True, stop=True,
            )
            dst = ysb1[:, tt, oc2, :]
            vec_or_pool().tensor_add(dst, acc[:], dst)

        def pjb_go(tt, oc2):
            force(("pj2", tt, oc2))
            ysb = worky.tile([P, 512], DT_BF, tag="ysb")
            acc = proj_acc()
            nc.tensor.matmul(
                acc[:],
                oT[:, 3, tt * P:(tt + 1) * P],
                wp[:, 3, oc2 * 512:(oc2 + 1) * 512],
                start=True, stop=True,
            )
            # alternate the final adds across DVE and GpSimd so neither
            # engine's in-order queue serializes the tail
            vec_or_pool().tensor_add(ysb[:], acc[:], ysb1[:, tt, oc2, :])
            # issue from ACT's DGE (idle during the proj tail: all head-7
            # exps were precomputed), keeping the SP DMA queue free for the
            # O transposes — in-order SEQ head-of-line blocking otherwise
            # serializes the whole tail
            nc.scalar.dma_start(
                y[tt * P:(tt + 1) * P, oc2 * 512:(oc2 + 1) * 512], ysb[:])

        def st7_go(jt):
            # head-7 S.T row jt computed & exp'd early into pt7 (ACT filler)
            if jt < 4:
                # these rows alias xt0: every consumer of x must be emitted
                # first (no-ops once they already ran)
                for oc in range(8):
                    for tc4 in range(4):
                        force(("qk", oc, tc4))
                for tt in range(NT):
                    for half in range(2):
                        force(("v", tt, half))
            dst, off = pt7row(jt)
            w = T - jt * P
            pos = 0
            while pos < w:
                big = state["flip"] == 0
                state["flip"] ^= 1
                cw = min(1024 if big else 512, w - pos)
                st_exp(NH - 1, jt, pos, cw, big, dst[:, off + pos:off + pos + cw])
                pos += cw

        UNITS = {}
        for oc in range(8):
            for tc4 in range(4):
                UNITS[("qk", oc, tc4)] = (4096, lambda o=oc, t=tc4: qk_go(o, t))
        for tt in range(NT):
            for half in range(2):
                UNITS[("v", tt, half)] = (2048, lambda t=tt, h=half: v_go(t, h))
        for tt in range(NT):
            for oc2 in range(2):
                UNITS[("pja", tt, oc2)] = (1024,
                                           lambda t=tt, o=oc2: pja_go(t, o))
                UNITS[("pj2", tt, oc2)] = (512,
                                           lambda t=tt, o=oc2: pj2_go(t, o))
                UNITS[("pjb", tt, oc2)] = (512,
                                           lambda t=tt, o=oc2: pjb_go(t, o))
        for jt in range(NT):
            UNITS[("st7", jt)] = (0, lambda j=jt: st7_go(j))  # self-paced

        # ---- emission-time pacing state ----
        state = {"pe": 0.0, "act": 0.0, "flip": 0, "tidx": 0, "vp": 0}
        emitted = set()
        fillers = []          # pure-PE unit keys, FIFO
        act_fillers = []      # ACT-heavy unit keys (early head-7 rows)

        def force(key):
            if key in emitted:
                return
            emitted.add(key)
            cyc, fn = UNITS[key]
            fn()
            state["pe"] += cyc * _PE_CYC

        def pace_fillers(slack=1500.0):
            while fillers:
                if fillers[0] in emitted:
                    fillers.pop(0)
                    continue
                cyc = UNITS[fillers[0]][0]
                if state["pe"] + cyc * _PE_CYC <= state["act"] + slack:
                    force(fillers.pop(0))
                else:
                    break

        # ---- attention ----
        norm_pending = []

        def emit_norms():
            while norm_pending:
                h, jt = norm_pending.pop(0)
                po = 64 * (h % 2)
                pr = h // 2
                avs = av_slot(jt)
                rec = recp.tile([P, 1], DT_F32, tag="rec")
                nc.vector.reciprocal_approx_fast(rec[:], avs[:, 64:65])
                nc.gpsimd.tensor_scalar_mul(
                    ostg[:, jt, po:po + 64], avs[:, 0:64], rec[:])
                if h % 2 == 1 and h < NH - 1:
                    # both heads of the pair staged: transpose [t,d] -> oT[d,t]
                    nc.sync.dma_start_transpose(
                        oT[:, pr, jt * P:(jt + 1) * P], ostg[:, jt, :])
                elif h % 2 == 1:
                    # pair 3 feeds the proj tail directly: a DMA transpose
                    # costs ~5us of latency per block, so transpose on the
                    # PE (into the freed stB bank) + DVE copy instead
                    tp = tail_slots_tp[jt % 2][:, 0:P]
                    nc.tensor.transpose(tp, ostg[:, jt, :], ident[:])
                    nc.vector.tensor_copy(oT[:, pr, jt * P:(jt + 1) * P], tp)
                    state["pe"] += P * _PE_CYC

        def st_exp(h, jt, pos, cw, big, dst):
            """S.T psum chunk for (head h, key-row jt, cols pos..pos+cw) with
            diagonal mask, exp'd into dst (bf16 sbuf)."""
            pr, po = h // 2, 64 * (h % 2)
            i0 = jt * P + pos
            for tc4 in range(i0 // 512, (i0 + cw + 511) // 512):
                force(("qk", pr, tc4))                   # Q cols producers
            st = (stA if big else stB)[:, 0:cw]
            for s0 in range(0, cw, 512):
                sw = min(512, cw - s0)
                nc.tensor.matmul(
                    st[:, s0:s0 + sw],
                    qkT[:, 4 + pr, :][po:po + 64, jt * P:(jt + 1) * P],
                    qkT[:, pr, :][po:po + 64, i0 + s0:i0 + s0 + sw],
                    start=True, stop=(False if s0 == 0 and pos == 0 else True),
                    skip_group_check=True,
                )
            if pos == 0:
                # causal mask add on the diagonal block, on the PE:
                # psum[0:128] += ident.T @ maskT (exactly maskT)
                nc.tensor.matmul(
                    st[:, 0:P], ident[:], maskT[:],
                    start=False, stop=True, skip_group_check=True,
                )
            nc.scalar.activation(dst, st[:], Exp)
            state["pe"] += (cw + (P if pos == 0 else 0)) * _PE_CYC
            state["act"] = max(state["act"], state["pe"]) \
                + cw * _ACT_CYC + _ACT_OVH

        def emit_head(h):
            # reset pacing cursors per head: estimates drift from the actual
            # simulator over time; within-head relative balance is what counts
            state["pe"] = 0.0
            state["act"] = 0.0
            pr = h // 2
            vhalf = 0 if h < 4 else 1
            if h == NH - 1:
                # all of head 7's P.T is materialized before its AV phase;
                # the S.T psum banks are then free to widen the proj acc
                # rotation for the tail
                for jt in range(NT):
                    force(("st7", jt))
                tailA = st_ps.tile([P, 1024], DT_F32, tag="stA")
                tailB = st_ps.tile([P, 512], DT_F32, tag="stB")
                tail_slots.extend(
                    [None, tailA[:, 0:512], None, tailA[:, 512:1024]])
                # two bf16 [128,128] psum regions in stB for PE transposes
                tail_slots_tp.extend(
                    [tailB[:, 0:64].bitcast(DT_BF),
                     tailB[:, 64:128].bitcast(DT_BF)])
            for jt in range(NT):
                force(("qk", 4 + pr, jt // 4))           # K block producer
                force(("v", jt, vhalf))                  # V producer for AV
                # early head-7 rows ride along where ACT has slack (the
                # PE-bound heads); rows 0-3 (aliasing xt0) go after all x
                # consumers have run
                st7_marks = {
                    1: [(3, 4), (9, 5)],
                    2: [(2, 6), (7, 7), (12, 8)],
                    3: [(2, 9), (7, 10), (12, 11)],
                    4: [(2, 12), (5, 13), (8, 14), (10, 15), (13, 0)],
                    5: [(2, 1), (7, 2), (12, 3)],
                }
                for mjt, row in st7_marks.get(h, []):
                    if jt == mjt:
                        force(("st7", row))
                w = T - jt * P
                if h == NH - 1:
                    # P.T for this row was computed early; AV-only row
                    emit_norms()
                    dst, off = pt7row(jt)
                    for k in range(w // P):
                        it = jt + k
                        nc.tensor.matmul(
                            av_slot(it),
                            dst[:, off + k * P:off + (k + 1) * P],
                            v65[:, jt, h, :],
                            start=(jt == 0), stop=(jt == it),
                            skip_group_check=True,
                        )
                    state["pe"] += (w // P) * 65 * _PE_CYC
                    # no exps here -> act cursor never advances; interleave
                    # the transpose-free proj fillers by fiat
                    for _ in range(2):
                        if fillers:
                            key = fillers.pop(0)
                            if key not in emitted:
                                force(key)
                else:
                    pos = 0
                    while pos < w:
                        big = state["flip"] == 0
                        state["flip"] ^= 1
                        cw = min(1024 if big else 512, w - pos)
                        pt = ptp.tile([P, 1024], DT_BF, tag="pt")
                        st_exp(h, jt, pos, cw, big, pt[:, 0:cw])
                        if pos == 0:
                            emit_norms()
                        for k in range(cw // P):
                            it = jt + pos // P + k
                            nc.tensor.matmul(
                                av_slot(it),
                                pt[:, k * P:(k + 1) * P],
                                v65[:, jt, h, :],
                                start=(jt == 0), stop=(jt == it),
                                skip_group_check=True,
                            )
                        state["pe"] += (cw // P) * 65 * _PE_CYC
                        pos += cw
                        pace_fillers()
                norm_pending.append((h, jt))
            emit_norms()

        # ---- schedule ----
        # filler queues per head, ~24k PE cycles each (the per-head PE
        # deficit vs ACT), honoring hard deadlines (pair p before head 2p,
        # v half-1 before head 4, pja after head 3).
        filler_plan = {
            0: [("qk", 0, 2), ("qk", 0, 3), ("qk", 4, 1),
                ("qk", 4, 2), ("qk", 4, 3)],
            1: [("qk", 5, 0), ("qk", 1, 0), ("qk", 1, 1), ("qk", 1, 2),
                ("qk", 1, 3), ("qk", 5, 1), ("qk", 5, 2), ("qk", 5, 3)],
            2: [("v", tt, 1) for tt in range(NT)],
            3: [("qk", 6, 0), ("qk", 2, 0), ("qk", 2, 1), ("qk", 2, 2),
                ("qk", 2, 3), ("qk", 6, 1), ("qk", 6, 2), ("qk", 6, 3)],
            4: [("qk", 7, 0), ("qk", 3, 0), ("qk", 3, 1), ("qk", 3, 2),
                ("qk", 3, 3), ("qk", 7, 1), ("qk", 7, 2), ("qk", 7, 3)],
            5: [("pja", tt, oc2) for tt in range(NT) for oc2 in range(2)],
            6: [("pj2", tt, oc2) for tt in range(NT) for oc2 in range(2)],
        }
        # pre-phase: warm PE while x streams in; head 0 needs these anyway
        # (all depend only on the first two x quarters)
        force(("qk", 4, 0))
        force(("qk", 0, 0))
        force(("qk", 0, 1))
        force(("v", 0, 0))
        force(("qk", 4, 1))
        force(("v", 1, 0))
        for h in range(NH):
            fillers.extend(filler_plan.get(h, []))
            emit_head(h)
        for key in list(fillers):
            if key not in emitted:
                force(key)
        for tt in range(NT):
            for oc2 in range(2):
                force(("pjb", tt, oc2))

    nc.compile()
    return nc


def _prep_inputs(x, W_qkv, W_proj):
    """Per-core host-side sharding and layout prep."""
    bf16 = ml_dtypes.bfloat16
    scale = np.float32(HD ** -0.5)
    in_maps = []
    for c in range(NCORES):
        b, hg = c // 2, c % 2
        heads = list(range(hg * 8, hg * 8 + 8))
        rq = np.concatenate([np.arange(h * 192, h * 192 + 64) for h in heads])
        rk = np.concatenate([np.arange(h * 192 + 64, h * 192 + 128) for h in heads])
        rv = np.concatenate([np.arange(h * 192 + 128, h * 192 + 192) for h in heads])
        wq = W_qkv[rq] * scale           # fold softmax scale into Q (exact: /8)
        wk = W_qkv[rk]
        wqkT = np.ascontiguousarray(np.concatenate([wq, wk], 0).T).astype(bf16)
        wvT = np.ascontiguousarray(W_qkv[rv].T).astype(bf16)
        wpT = np.ascontiguousarray(W_proj[:, hg * 512:(hg + 1) * 512].T)
        xTb = np.ascontiguousarray(x[b].T).astype(bf16)
        in_maps.append({"xT": xTb, "wqkT": wqkT, "wvT": wvT,
                        "wpT": wpT.astype(bf16)})
    return in_maps


def kernel(x, W_qkv, W_proj, b_proj):
    from concourse.bass_utils import run_bass_kernel_spmd

    x = np.asarray(x, dtype=np.float32)
    W_qkv = np.asarray(W_qkv, dtype=np.float32)
    W_proj = np.asarray(W_proj, dtype=np.float32)
    b_proj = np.asarray(b_proj, dtype=np.float32)

    if "nc" not in _CACHE:
        _CACHE["nc"] = _build_program()
    nc = _CACHE["nc"]

    in_maps = _prep_inputs(x, W_qkv, W_proj)
    res = run_bass_kernel_spmd(nc, in_maps, core_ids=list(range(NCORES)))
    out = np.empty((B, T, C), dtype=np.float32)
    for b in range(B):
        out[b] = (res.results[2 * b]["y"].astype(np.float32)
                  + res.results[2 * b + 1]["y"].astype(np.float32) + b_proj)
    return out


# revision 78
# speedup vs baseline: 1.0239x; 1.0239x over previous
"""Multi-head causal attention (B=4, T=2048, C=1024, H=16, HD=64) on 8 TRN2 NeuronCores.

Sharding: core c handles batch b = c//2 and heads hg*8..hg*8+8 where hg = c%2
(data parallel on B, tensor parallel on heads). Each core computes
qkv projection for its head group, causal attention for its 8 heads, and a
partial output projection over its 512 local channels. Host sums the two
partial projections per batch and adds the bias.

Attention is computed transposed (S.T = K.T-stationary @ Q-moving) so that
softmax'd probabilities P.T come out of the exp directly in the layout the
AV product wants as its *stationary* operand:
  - per (head, key-row-block jt): S.T[j, i] for i >= jt*128 in psum chunks,
    causal mask added on the diagonal 128-block by GpSimd, exp on ACT into
    bf16 P.T chunks (no transposes anywhere).
  - AV: out[i-tile, 65] += P.T-block (stationary) @ [V_head | 1] (moving)
    accumulated over jt in 16 per-head psum accumulators; the ones column
    yields the softmax row-sums for free.
  - normalize: 1/rowsum (DVE) * O (GpSimd) -> bf16, DMA-transposed into
    oT[d, t] for the output projection.
qkv projection / output projection matmul groups are interleaved between
attention chunks as PE filler, paced by an emission-time cost estimate, so
the PE never starves while ACT drains the exps. Same-engine producers are
force-emitted before their consumers (engines execute in order).

Per-core device layouts (all chosen so no on-chip transposes of x/W needed):
  xT    [C=1024, T=2048] bf16   (x[b].T, host-transposed)
  wqkT  [C=1024, 1024]   bf16   (rows: Q of 8 heads then K of 8 heads, transposed;
                                 Q part pre-scaled by HD^-0.5)
  wvT   [C=1024, 512]    bf16
  wpT   [512, 1024]      bf16   (W_proj columns for local channels, transposed)
"""

import numpy as np
import ml_dtypes

B, T, C = 4, 2048, 1024
H = 16
HD = 64
NCORES = 8
P = 128

_CACHE = {}

# cost-model constants for emission-time pacing (ns)
_PE_CYC = 1.0 / 2.4
_ACT_CYC = 1.0 / 1.2
_ACT_OVH = 250.0


def _build_program():
    import concourse.bass as bass
    import concourse.mybir as mybir
    import concourse.tile as tile
    from concourse import bacc
    from concourse.masks import make_lower_triangular, make_identity
    from contextlib import ExitStack

    DT_BF = mybir.dt.bfloat16
    DT_F32 = mybir.dt.float32
    Exp = mybir.ActivationFunctionType.Exp

    nc = bacc.Bacc("TRN2", target_bir_lowering=False, num_devices=NCORES)
    xT = nc.dram_tensor("xT", [C, T], DT_BF, kind="ExternalInput")
    wqkT = nc.dram_tensor("wqkT", [C, 1024], DT_BF, kind="ExternalInput")
    wvT = nc.dram_tensor("wvT", [C, 512], DT_BF, kind="ExternalInput")
    wpT = nc.dram_tensor("wpT", [512, 1024], DT_BF, kind="ExternalInput")
    # y is bf16: halves the store traffic; the host sums the two per-batch
    # partials in fp32 (the partials themselves are bf16-rounded anyway)
    y = nc.dram_tensor("y", [T, C], DT_BF, kind="ExternalOutput")

    NT = T // P            # 16 t-tiles
    NH = 8                 # heads per core

    with tile.TileContext(nc) as tc, ExitStack() as ctx:
        pers = ctx.enter_context(tc.tile_pool(name="pers", bufs=1))
        ptp = ctx.enter_context(tc.tile_pool(name="ptp", bufs=3))
        recp = ctx.enter_context(tc.tile_pool(name="recp", bufs=4))
        worky = ctx.enter_context(tc.tile_pool(name="worky", bufs=8))
        # PSUM banks: stA 2 + stB 1 + av 3 + acc 2 = 8
        st_ps = ctx.enter_context(tc.tile_pool(name="st", bufs=1, space="PSUM"))
        av_ps = ctx.enter_context(tc.tile_pool(name="av", bufs=1, space="PSUM"))
        acc_ps = ctx.enter_context(tc.tile_pool(name="acc", bufs=2, space="PSUM"))

        # ---- persistent tiles ----
        xt0 = pers.tile([P, 4, T], DT_BF, tag="xt0")
        xt1 = pers.tile([P, 4, T], DT_BF, tag="xt1")

        def xt(cc):
            return (xt0 if cc < 4 else xt1)[:, cc % 4, :]

        wqk = pers.tile([P, 8, 1024], DT_BF, tag="wqk")
        wv = pers.tile([P, 8, 512], DT_BF, tag="wv")
        wp = pers.tile([P, 4, 1024], DT_BF, tag="wp")
        qkT = pers.tile([P, 8, T], DT_BF, tag="qkT")
        v65 = pers.tile([P, NT, NH, 65], DT_BF, tag="v65")
        oT = pers.tile([P, 4, T], DT_BF, tag="oT")
        # normalized-O staging for the in-flight head pair: [t, it, d-pair]
        ostg = pers.tile([P, NT, P], DT_BF, tag="ostg")
        # causal mask + identity (bf16): the mask add rides the PE as
        # ident.T @ maskT accumulated into the S.T psum diagonal block
        maskT = pers.tile([P, P], DT_BF, tag="maskT")
        ident = pers.tile([P, P], DT_BF, tag="ident")
        # head 7's P.T rows, ALL exp'd early (during heads with ACT slack) so
        # the last head needs no ACT at all. Rows 4..15 live in their own
        # tile; rows 0..3 (the big ones) alias xt0's slot, which is dead once
        # every qkv-projection group has run.
        PT7D_OFF = [0]
        for j in range(5, NT):
            PT7D_OFF.append(PT7D_OFF[-1] + T - (j - 1) * P)
        pt7d = pers.tile([P, PT7D_OFF[-1] + P], DT_BF, tag="pt7d")
        pt7a_cache = []

        def pt7row(jt):
            if jt < 4:
                if not pt7a_cache:
                    # one-time realias of xt0's slot (allocated once so all
                    # P.T writes/reads refer to the same logical tile)
                    a = pers.tile([P, 4, T], DT_BF, tag="xt0")
                    pt7a_cache.append(a[:].rearrange("p a t -> p (a t)"))
                return pt7a_cache[0], jt * T - P * (jt * (jt - 1) // 2)
            return pt7d[:], PT7D_OFF[jt - 4]

        stA = st_ps.tile([P, 1024], DT_F32, tag="stA")
        stB = st_ps.tile([P, 512], DT_F32, tag="stB")
        avA = av_ps.tile([P, 7, 65], DT_F32, tag="avA")
        avB = av_ps.tile([P, 7, 65], DT_F32, tag="avB")
        avC = av_ps.tile([P, 2, 65], DT_F32, tag="avC")

        def av_slot(it):
            if it < 7:
                return avA[:, it, :]
            if it < 14:
                return avB[:, it - 7, :]
            return avC[:, it - 14, :]

        # ---- loads ----
        xtre = xT.rearrange("(o p) t -> p o t", p=P)
        wqkre = wqkT.rearrange("(o p) f -> p o f", p=P)
        wvre = wvT.rearrange("(o p) f -> p o f", p=P)
        wpre = wpT.rearrange("(o p) f -> p o f", p=P)
        # weights for pair-0 q/k first, then x in halves; few big DMAs (HWDGE
        # issue costs ~630ns apiece, so batch)
        # NB: qkv matmul chains contract over all 8 c-chunks, so weight loads
        # are split by OUTPUT columns (w*[:, :, osl]), never by c-chunk.
        nc.sync.dma_start(wqk[:, :, 512:768], wqkre[:, :, 512:768])    # oc 4,5
        for tq in range(4):
            tsl = slice(tq * 512, (tq + 1) * 512)
            nc.sync.dma_start(xt0[:, :, tsl], xtre[:, 0:4, tsl])
            nc.sync.dma_start(xt1[:, :, tsl], xtre[:, 4:8, tsl])
            if tq == 0:
                nc.sync.dma_start(wqk[:, :, 0:256], wqkre[:, :, 0:256])  # 0,1
            if tq == 1:
                nc.sync.dma_start(wv[:, :, 0:256], wvre[:, :, 0:256])
        nc.sync.dma_start(wqk[:, :, 768:1024], wqkre[:, :, 768:1024])  # oc 6,7
        nc.sync.dma_start(wqk[:, :, 256:512], wqkre[:, :, 256:512])    # oc 2,3
        nc.sync.dma_start(wv[:, :, 256:512], wvre[:, :, 256:512])
        nc.sync.dma_start(wp[:, :, :], wpre[:, 0:4, :])

        # S.T-layout causal mask: -1e30 where j > i (strict lower triangle)
        make_lower_triangular(nc, maskT[:], val=-1e30, diag=False)
        make_identity(nc, ident[:])
        # ones column of [V | 1] (rowsum accumulator input)
        nc.vector.memset(v65[:, :, :, 64:65], 1.0)

        # ---- work units: qkv projection / output projection groups ----
        def qk_go(oc, tc4):
            ts = slice(tc4 * 512, (tc4 + 1) * 512)
            acc = acc_ps.tile([P, 512], DT_F32, tag="acc")
            for cc in range(8):
                nc.tensor.matmul(
                    acc[:],
                    wqk[:, cc, oc * P:(oc + 1) * P],
                    xt(cc)[:, ts],
                    start=(cc == 0), stop=(cc == 7),
                )
            nc.vector.tensor_copy(qkT[:, oc, ts], acc[:])

        def v_go(tt, half):
            acc = acc_ps.tile([P, 512], DT_F32, tag="acc")
            for cc in range(8):
                nc.tensor.matmul(
                    acc[:, 0:256],
                    xt(cc)[:, tt * P:(tt + 1) * P],
                    wv[:, cc, half * 256:(half + 1) * 256],
                    start=(cc == 0), stop=(cc == 7),
                )
            src = acc[:, 0:256].rearrange("p (h c) -> p h c", c=64)
            nc.vector.tensor_copy(v65[:, tt, half * 4:(half + 1) * 4, 0:64], src)

        # output projection in two per-pair halves: pairs {0,1} become
        # available after head 3, so they fill the late-middle heads; pairs
        # {2,3} complete during head 7. Partial sums staged in bf16 SBUF.
        ysb1 = pers.tile([P, NT, 2, 512], DT_BF, tag="ysb1")
        tail_slots = []
        tail_slots_tp = []

        def proj_acc():
            """PSUM acc for a projection unit; in the tail (head 7 onward:
            no S.T chunks anymore) rotate over the freed st banks too, so
            the psum-readback latency doesn't serialize the proj stream."""
            if tail_slots:
                state["tidx"] += 1
                sl = tail_slots[state["tidx"] % len(tail_slots)]
                if sl is not None:
                    return sl
            return acc_ps.tile([P, 512], DT_F32, tag="acc", name="acc")

        def vec_or_pool():
            state["vp"] ^= 1
            return nc.vector if state["vp"] else nc.gpsimd

        def pja_go(tt, oc2):
            acc = proj_acc()
            for hc in range(2):
                nc.tensor.matmul(
                    acc[:],
                    oT[:, hc, tt * P:(tt + 1) * P],
                    wp[:, hc, oc2 * 512:(oc2 + 1) * 512],
                    start=(hc == 0), stop=(hc == 1),
                )
            vec_or_pool().tensor_copy(ysb1[:, tt, oc2, :], acc[:])

        def pj2_go(tt, oc2):
            force(("pja", tt, oc2))
            acc = proj_acc()
            nc.tensor.matmul(
                acc[:],
                oT[:, 2, tt * P:(tt + 1) * P],
                wp[:, 2, oc2 * 512:(oc2 + 1) * 512],
                start=True, stop=True,
            )
            dst = ysb1[:, tt, oc2, :]
            vec_or_pool().tensor_add(dst, acc[:], dst)

        def pjb_go(tt, oc2):
            force(("pj2", tt, oc2))
            ysb = worky.tile([P, 512], DT_BF, tag="ysb")
            acc = proj_acc()
            nc.tensor.matmul(
                acc[:],
                oT[:, 3, tt * P:(tt + 1) * P],
                wp[:, 3, oc2 * 512:(oc2 + 1) * 512],
                start=True, stop=True,
            )
            # alternate the final adds across DVE and GpSimd so neither
            # engine's in-order queue serializes the tail
            vec_or_pool().tensor_add(ysb[:], acc[:], ysb1[:, tt, oc2, :])
            # issue from ACT's DGE (idle during the proj tail: all head-7
            # exps were precomputed), keeping the SP DMA queue free for the
            # O transposes — in-order SEQ head-of-line blocking otherwise
            # serializes the whole tail
            nc.scalar.dma_start(
                y[tt * P:(tt + 1) * P, oc2 * 512:(oc2 + 1) * 512], ysb[:])

        def st7_go(jt):
            # head-7 S.T row jt computed & exp'd early into pt7 (ACT filler)
            if jt < 4:
                # these rows alias xt0: every consumer of x must be emitted
                # first (no-ops once they already ran)
                for oc in range(8):
                    for tc4 in range(4):
                        force(("qk", oc, tc4))
                for tt in range(NT):
                    for half in range(2):
                        force(("v", tt, half))
            dst, off = pt7row(jt)
            w = T - jt * P
            pos = 0
            while pos < w:
                big = state["flip"] == 0
                state["flip"] ^= 1
                cw = min(1024 if big else 512, w - pos)
                st_exp(NH - 1, jt, pos, cw, big, dst[:, off + pos:off + pos + cw])
                pos += cw

        UNITS = {}
        for oc in range(8):
            for tc4 in range(4):
                UNITS[("qk", oc, tc4)] = (4096, lambda o=oc, t=tc4: qk_go(o, t))
        for tt in range(NT):
            for half in range(2):
                UNITS[("v", tt, half)] = (2048, lambda t=tt, h=half: v_go(t, h))
        for tt in range(NT):
            for oc2 in range(2):
                UNITS[("pja", tt, oc2)] = (1024,
                                           lambda t=tt, o=oc2: pja_go(t, o))
                UNITS[("pj2", tt, oc2)] = (512,
                                           lambda t=tt, o=oc2: pj2_go(t, o))
                UNITS[("pjb", tt, oc2)] = (512,
                                           lambda t=tt, o=oc2: pjb_go(t, o))
        for jt in range(NT):
            UNITS[("st7", jt)] = (0, lambda j=jt: st7_go(j))  # self-paced

        # ---- emission-time pacing state ----
        state = {"pe": 0.0, "act": 0.0, "flip": 0, "tidx": 0, "vp": 0}
        emitted = set()
        fillers = []          # pure-PE unit keys, FIFO
        act_fillers = []      # ACT-heavy unit keys (early head-7 rows)

        def force(key):
            if key in emitted:
                return
            emitted.add(key)
            cyc, fn = UNITS[key]
            fn()
            state["pe"] += cyc * _PE_CYC

        def pace_fillers(slack=300.0):
            while fillers:
                if fillers[0] in emitted:
                    fillers.pop(0)
                    continue
                cyc = UNITS[fillers[0]][0]
                if state["pe"] + cyc * _PE_CYC <= state["act"] + slack:
                    force(fillers.pop(0))
                else:
                    break

        # ---- attention ----
        norm_pending = []

        def emit_norms():
            while norm_pending:
                h, jt = norm_pending.pop(0)
                po = 64 * (h % 2)
                pr = h // 2
                avs = av_slot(jt)
                rec = recp.tile([P, 1], DT_F32, tag="rec")
                nc.vector.reciprocal_approx_fast(rec[:], avs[:, 64:65])
                nc.gpsimd.tensor_scalar_mul(
                    ostg[:, jt, po:po + 64], avs[:, 0:64], rec[:])
                if h % 2 == 1 and h < NH - 1:
                    # both heads of the pair staged: transpose [t,d] -> oT[d,t]
                    nc.sync.dma_start_transpose(
                        oT[:, pr, jt * P:(jt + 1) * P], ostg[:, jt, :])
                elif h % 2 == 1:
                    # pair 3 feeds the proj tail directly: a DMA transpose
                    # costs ~5us of latency per block, so transpose on the
                    # PE (into the freed stB bank) + DVE copy instead
                    tp = tail_slots_tp[jt % 2][:, 0:P]
                    nc.tensor.transpose(tp, ostg[:, jt, :], ident[:])
                    nc.vector.tensor_copy(oT[:, pr, jt * P:(jt + 1) * P], tp)
                    state["pe"] += P * _PE_CYC

        def st_exp(h, jt, pos, cw, big, dst):
            """S.T psum chunk for (head h, key-row jt, cols pos..pos+cw) with
            diagonal mask, exp'd into dst (bf16 sbuf)."""
            pr, po = h // 2, 64 * (h % 2)
            i0 = jt * P + pos
            for tc4 in range(i0 // 512, (i0 + cw + 511) // 512):
                force(("qk", pr, tc4))                   # Q cols producers
            st = (stA if big else stB)[:, 0:cw]
            for s0 in range(0, cw, 512):
                sw = min(512, cw - s0)
                nc.tensor.matmul(
                    st[:, s0:s0 + sw],
                    qkT[:, 4 + pr, :][po:po + 64, jt * P:(jt + 1) * P],
                    qkT[:, pr, :][po:po + 64, i0 + s0:i0 + s0 + sw],
                    start=True, stop=(False if s0 == 0 and pos == 0 else True),
                    skip_group_check=True,
                )
            if pos == 0:
                # causal mask add on the diagonal block, on the PE:
                # psum[0:128] += ident.T @ maskT (exactly maskT)
                nc.tensor.matmul(
                    st[:, 0:P], ident[:], maskT[:],
                    start=False, stop=True, skip_group_check=True,
                )
            nc.scalar.activation(dst, st[:], Exp)
            state["pe"] += (cw + (P if pos == 0 else 0)) * _PE_CYC
            state["act"] = max(state["act"], state["pe"]) \
                + cw * _ACT_CYC + _ACT_OVH

        def emit_head(h):
            # reset pacing cursors per head: estimates drift from the actual
            # simulator over time; within-head relative balance is what counts
            state["pe"] = 0.0
            state["act"] = 0.0
            pr = h // 2
            vhalf = 0 if h < 4 else 1
            if h == NH - 1:
                # all of head 7's P.T is materialized before its AV phase;
                # the S.T psum banks are then free to widen the proj acc
                # rotation for the tail
                for jt in range(NT):
                    force(("st7", jt))
                tailA = st_ps.tile([P, 1024], DT_F32, tag="stA")
                tailB = st_ps.tile([P, 512], DT_F32, tag="stB")
                tail_slots.extend(
                    [None, tailA[:, 0:512], None, tailA[:, 512:1024]])
                # two bf16 [128,128] psum regions in stB for PE transposes
                tail_slots_tp.extend(
                    [tailB[:, 0:64].bitcast(DT_BF),
                     tailB[:, 64:128].bitcast(DT_BF)])
            for jt in range(NT):
                force(("qk", 4 + pr, jt // 4))           # K block producer
                force(("v", jt, vhalf))                  # V producer for AV
                # early head-7 rows ride along where ACT has slack (the
                # PE-bound heads); rows 0-3 (aliasing xt0) go after all x
                # consumers have run
                st7_marks = {
                    1: [(3, 4), (9, 5)],
                    2: [(2, 6), (7, 7), (12, 8)],
                    3: [(2, 9), (7, 10), (12, 11)],
                    4: [(2, 12), (5, 13), (8, 14), (10, 15), (13, 0)],
                    5: [(2, 1), (7, 2), (12, 3)],
                }
                for mjt, row in st7_marks.get(h, []):
                    if jt == mjt:
                        force(("st7", row))
                w = T - jt * P
                if h == NH - 1:
                    # P.T for this row was computed early; AV-only row
                    emit_norms()
                    dst, off = pt7row(jt)
                    for k in range(w // P):
                        it = jt + k
                        nc.tensor.matmul(
                            av_slot(it),
                            dst[:, off + k * P:off + (k + 1) * P],
                            v65[:, jt, h, :],
                            start=(jt == 0), stop=(jt == it),
                            skip_group_check=True,
                        )
                    state["pe"] += (w // P) * 65 * _PE_CYC
                    # no exps here -> act cursor never advances; interleave
                    # the transpose-free proj fillers by fiat
                    for _ in range(2):
                        if fillers:
                            key = fillers.pop(0)
                            if key not in emitted:
                                force(key)
                else:
                    pos = 0
                    while pos < w:
                        big = state["flip"] == 0
                        state["flip"] ^= 1
                        cw = min(1024 if big else 512, w - pos)
                        pt = ptp.tile([P, 1024], DT_BF, tag="pt")
                        st_exp(h, jt, pos, cw, big, pt[:, 0:cw])
                        if pos == 0:
                            emit_norms()
                        for k in range(cw // P):
                            it = jt + pos // P + k
                            nc.tensor.matmul(
                                av_slot(it),
                                pt[:, k * P:(k + 1) * P],
                                v65[:, jt, h, :],
                                start=(jt == 0), stop=(jt == it),
                                skip_group_check=True,
                            )
                        state["pe"] += (cw // P) * 65 * _PE_CYC
                        pos += cw
                        pace_fillers()
                norm_pending.append((h, jt))
            emit_norms()

        # ---- schedule ----
        # filler queues per head, ~24k PE cycles each (the per-head PE
        # deficit vs ACT), honoring hard deadlines (pair p before head 2p,
        # v half-1 before head 4, pja after head 3).
        filler_plan = {
            0: [("qk", 0, 2), ("qk", 0, 3), ("qk", 4, 1),
                ("qk", 4, 2), ("qk", 4, 3)],
            1: [("qk", 5, 0), ("qk", 1, 0), ("qk", 1, 1), ("qk", 1, 2),
                ("qk", 1, 3), ("qk", 5, 1), ("qk", 5, 2), ("qk", 5, 3)],
            2: [("v", tt, 1) for tt in range(NT)],
            3: [("qk", 6, 0), ("qk", 2, 0), ("qk", 2, 1), ("qk", 2, 2),
                ("qk", 2, 3), ("qk", 6, 1), ("qk", 6, 2), ("qk", 6, 3)],
            4: [("qk", 7, 0), ("qk", 3, 0), ("qk", 3, 1), ("qk", 3, 2),
                ("qk", 3, 3), ("qk", 7, 1), ("qk", 7, 2), ("qk", 7, 3)],
            5: [("pja", tt, oc2) for tt in range(NT) for oc2 in range(2)],
            6: [("pj2", tt, oc2) for tt in range(NT) for oc2 in range(2)],
        }
        # pre-phase: warm PE while x streams in; head 0 needs these anyway
        # (all depend only on the first two x quarters)
        force(("qk", 4, 0))
        force(("qk", 0, 0))
        force(("qk", 0, 1))
        force(("v", 0, 0))
        force(("qk", 4, 1))
        force(("v", 1, 0))
        for h in range(NH):
            fillers.extend(filler_plan.get(h, []))
            emit_head(h)
        for key in list(fillers):
            if key not in emitted:
                force(key)
        for tt in range(NT):
            for oc2 in range(2):
                force(("pjb", tt, oc2))

    nc.compile()
    return nc


def _prep_inputs(x, W_qkv, W_proj):
    """Per-core host-side sharding and layout prep."""
    bf16 = ml_dtypes.bfloat16
    scale = np.float32(HD ** -0.5)
    in_maps = []
    for c in range(NCORES):
        b, hg = c // 2, c % 2
        heads = list(range(hg * 8, hg * 8 + 8))
        rq = np.concatenate([np.arange(h * 192, h * 192 + 64) for h in heads])
        rk = np.concatenate([np.arange(h * 192 + 64, h * 192 + 128) for h in heads])
        rv = np.concatenate([np.arange(h * 192 + 128, h * 192 + 192) for h in heads])
        wq = W_qkv[rq] * scale           # fold softmax scale into Q (exact: /8)
        wk = W_qkv[rk]
        wqkT = np.ascontiguousarray(np.concatenate([wq, wk], 0).T).astype(bf16)
        wvT = np.ascontiguousarray(W_qkv[rv].T).astype(bf16)
        wpT = np.ascontiguousarray(W_proj[:, hg * 512:(hg + 1) * 512].T)
        xTb = np.ascontiguousarray(x[b].T).astype(bf16)
        in_maps.append({"xT": xTb, "wqkT": wqkT, "wvT": wvT,
                        "wpT": wpT.astype(bf16)})
    return in_maps


def kernel(x, W_qkv, W_proj, b_proj):
    from concourse.bass_utils import run_bass_kernel_spmd

    x = np.asarray(x, dtype=np.float32)
    W_qkv = np.asarray(W_qkv, dtype=np.float32)
    W_proj = np.asarray(W_proj, dtype=np.float32)
    b_proj = np.asarray(b_proj, dtype=np.float32)

    if "nc" not in _CACHE:
        _CACHE["nc"] = _build_program()
    nc = _CACHE["nc"]

    in_maps = _prep_inputs(x, W_qkv, W_proj)
    res = run_bass_kernel_spmd(nc, in_maps, core_ids=list(range(NCORES)))
    out = np.empty((B, T, C), dtype=np.float32)
    for b in range(B):
        out[b] = (res.results[2 * b]["y"].astype(np.float32)
                  + res.results[2 * b + 1]["y"].astype(np.float32) + b_proj)
    return out
